# revision 1
# baseline (speedup 1.0000x reference)
"""Trainium2 kernel for per-node multi-head neighbor attention (GNN message passing).

Reference computation (B=16384 nodes, N=32 neighbors, D=128, H=4 heads):
    q = x @ Wq_h^T ; k = nbr @ Wk_h^T ; v = nbr @ Wv_h^T
    logits = q k^T ; attn = softmax(logits) ; res = mean_h(attn @ v)
    out = leaky_relu(res @ Wo^T + bo)

Key optimization (makes the problem memory- instead of compute-bound):
fold the per-head projections into the tiny weight matrices once on the host:
    M_h = Wq_h^T @ Wk_h          => logits[e,h,n] = x[e] @ M_h @ nbr[e,n]^T
    U_h = (Wv_h^T @ Wo^T) / H    => out[e] = sum_h (attn[e,h] @ nbr[e]) @ U_h + bo
This removes the O(N*H*Dh*D) k/v projections per element (~7x less compute).

Sharding: pure data parallel over the batch dim across 8 NeuronCores.
"""

import numpy as np

B, N, D_IN, D_H, D_OUT, H = 16384, 32, 128, 128, 128, 4
N_CORES = 8

_COMPILED = {}


def _get_pmapped():
    if "fn" in _COMPILED:
        return _COMPILED["fn"]
    import jax
    import jax.numpy as jnp

    def shard_fn(x, nbr, M, U, bo):
        # x: [b, 128]   nbr: [b, 32, 128]   M: [H,128,128]  U: [H,128,128]
        qM = jnp.einsum("bi,hij->bhj", x, M)              # [b,H,128]
        logits = jnp.einsum("bhj,bnj->bhn", qM, nbr)      # [b,H,32]
        attn = jax.nn.softmax(logits, axis=-1)
        c = jnp.einsum("bhn,bnj->bhj", attn, nbr)         # [b,H,128]
        out = jnp.einsum("bhj,hjo->bo", c, U) + bo        # [b,128]
        return jax.nn.leaky_relu(out, negative_slope=0.01)

    fn = jax.pmap(shard_fn, axis_name="cores")
    _COMPILED["fn"] = fn
    return fn


def kernel(x, neighbors, Wq, Wk, Wv, Wo, bo):
    x = np.asarray(x, dtype=np.float32)
    neighbors = np.asarray(neighbors, dtype=np.float32)
    Wq = np.asarray(Wq, dtype=np.float32)
    Wk = np.asarray(Wk, dtype=np.float32)
    Wv = np.asarray(Wv, dtype=np.float32)
    Wo = np.asarray(Wo, dtype=np.float32)
    bo = np.asarray(bo, dtype=np.float32)

    # Host-side weight folding (tiny: 4 x 128^3 matmuls)
    M = np.einsum("hdi,hdj->hij", Wq, Wk).astype(np.float32)       # Wq_h^T @ Wk_h
    U = (np.einsum("hdi,od->hio", Wv, Wo) / H).astype(np.float32)  # Wv_h^T @ Wo^T / H

    bs = B // N_CORES
    xs = x[:, 0, :].reshape(N_CORES, bs, D_IN)
    nbrs = neighbors.reshape(N_CORES, bs, N, D_IN)
    Ms = np.broadcast_to(M, (N_CORES,) + M.shape)
    Us = np.broadcast_to(U, (N_CORES,) + U.shape)
    bos = np.broadcast_to(bo, (N_CORES, D_OUT))

    fn = _get_pmapped()
    out = fn(xs, nbrs, Ms, Us, bos)  # [8, bs, 128]
    return np.asarray(out).reshape(B, D_OUT).astype(np.float32)


if __name__ == "__main__":
    import reference

    inputs = reference.setup_inputs()
    inputs = {k: np.asarray(v) for k, v in inputs.items()}
    expected = np.asarray(reference.reference(**inputs))
    actual = kernel(**inputs)
    err = np.abs(actual - expected).max() / (np.abs(expected).max() + 1e-9)
    print("Relative error:", err)
